# revision 9
# baseline (speedup 1.0000x reference)
"""FASTMultiHeadAttention (polynomial softmax + RPE bias, causal) on 8 trn2 cores.

Math per (b,h):   s[i,j] = q_i.k_j + q_i.rpe[n-1+i-j]
                  score  = 1 + s + 0.5 s^2    (= 0.5[(s+1)^2 + 1], 0.5 cancels)
                  o_i    = sum_{j<=i} score v_j / sum_{j<=i} score

Device pipeline per (b,h)  [B*H = 32 units, 4 per core]:
  - m2r[i,u]  = q_i . rpeR[u]          (PE matmul, rpeR = reversed rpe band)
  - bias tile = shear-read of m2r      (SBUF->SBUF DMA, coupled access pattern)
  - psum_s    = qT.T @ kT              (PE)
  - w         = (psum_s + 1) + bias    (DVE scalar_tensor_tensor, evac to SBUF)
  - diag mask via gpsimd.affine_select
  - wT        = PE transpose(w)        -> psum (bf16)
  - scoreT    = Square(wT)             (ACT evac psum->SBUF)
  - oT[65,256] += vaug_J.T @ scoreT    (PE, accumulated over J; col 64 = ones)
  - oT evac to SBUF, DMA to HBM.

Host post-processing: add cumsum(v) correction (the a0=1 term) in f64, divide
by the denominator row, transpose back to [n, d].

s and m2r matmuls are K=64 row-packed onto the two 64-row halves of the PE
array (tile_position (0,0)/(64,0)); operands for the upper half live in
SBUF partitions 64-127.
"""

import sys

if "/opt/trn_rl_repo" not in sys.path:
    sys.path.insert(0, "/opt/trn_rl_repo")

import ml_dtypes
import numpy as np

import bass_rust
import concourse.bacc as bacc
import concourse.bass as bass
import concourse.mybir as mybir
import concourse.tile as tile
from concourse.bass_utils import run_bass_kernel_spmd

F32 = mybir.dt.float32
F32R = mybir.dt.float32r
BF16 = mybir.dt.bfloat16

B, H, N, D = 2, 16, 1024, 64
NBH = B * H  # 32
N_CORES = 8
BH_PER_CORE = NBH // N_CORES  # 4
NT = N // 128  # 8 row tiles
SROW = 1280  # m2r row-buffer width (elements)
RPE_W = 1152  # width of reversed rpe band (1151 + pad col)

ROW_PACK = True  # s/m2r on separate 64-row PE tiles

# Matmul chunks: exact causal widths, split at 512 (PSUM bank limit). All PE
# matmuls run in bf16 (1 cyc/col at any width on this part, FWL weight loads,
# and K=64 row-packing overlaps cleanly).
def _chunks(total):
    out = []
    c = 0
    while c < total:
        out.append((c, min(512, total - c)))
        c += 512
    return out


S_CHUNKS = {I: _chunks(128 * (I + 1)) for I in range(8)}
M2R_CHUNKS = {I: _chunks(255 + 128 * I) for I in range(8)}


def _shear_ap(t_ap, row_elems, offset, width):
    """AP reading t[p, offset - p + m] for m in [0, width)."""
    cp = t_ap.copy()
    cp.ap = bass_rust.VecI64Pair([[row_elems - 1, 128], [1, width]])
    cp.offset = offset
    return cp


def _ap(t_ap, pairs, offset=0):
    """Custom access pattern on a tile: pairs = [[step, count], ...] (elements)."""
    cp = t_ap.copy()
    cp.ap = bass_rust.VecI64Pair(pairs)
    cp.offset = offset
    return cp


def build_program():
    nc = bacc.Bacc(
        "TRN2", target_bir_lowering=False, debug=False, num_devices=N_CORES
    )

    qT_d = nc.dram_tensor("qT", [BH_PER_CORE, 64, N], BF16, kind="ExternalInput").ap()
    kT_d = nc.dram_tensor("kT", [BH_PER_CORE, 64, N], BF16, kind="ExternalInput").ap()
    va_d = nc.dram_tensor("va", [BH_PER_CORE, N, 65], BF16, kind="ExternalInput").ap()
    rpe_d = nc.dram_tensor("rpeR", [64, RPE_W], BF16, kind="ExternalInput").ap()
    idn_d = nc.dram_tensor("idn", [128, 128], BF16, kind="ExternalInput").ap()
    oT_d = nc.dram_tensor("oT", [BH_PER_CORE, 65, N], F32, kind="ExternalOutput").ap()

    with tile.TileContext(nc) as tc:
        with (
            tc.tile_pool(name="const", bufs=1) as cpool,
            tc.tile_pool(name="io", bufs=2) as io,
            tc.tile_pool(name="m2r", bufs=3) as m2rp,
            tc.tile_pool(name="wrow", bufs=3) as wp,
            tc.tile_pool(name="bias", bufs=4) as bp,
            tc.tile_pool(name="sct", bufs=2) as scp,
            tc.tile_pool(name="fin", bufs=2) as fp,
            tc.tile_pool(name="psms", bufs=2, space="PSUM") as ps_s,
            tc.tile_pool(name="psmr", bufs=2, space="PSUM") as ps_mr,
            tc.tile_pool(name="pswt", bufs=2, space="PSUM") as ps_wt,
            tc.tile_pool(name="psot", bufs=2, space="PSUM") as ps_ot,
        ):
            # constants; rpeR duplicated into partitions 64-127 for the
            # upper-half row-packed m2r matmuls. Startup DMAs are spread
            # across queues so the first row-tile's inputs land ASAP.
            rpeR = cpool.tile([128, RPE_W], BF16)
            nc.sync.dma_start(rpeR[64:128, :], rpe_d[:])
            nc.sync.dma_start(rpeR[0:64, :], rpe_d[:])
            idn = cpool.tile([128, 128], BF16)
            nc.scalar.dma_start(idn[:], idn_d[:])

            for m in range(BH_PER_CORE):
                # q is needed on both PE row-halves (s on rows 0-63, m2r on
                # 64-127)
                qT = io.tile([128, N], BF16, tag="qT")
                nc.gpsimd.dma_start(qT[64:128, :], qT_d[m])
                nc.gpsimd.dma_start(qT[0:64, :], qT_d[m])
                kT = io.tile([64, N], BF16, tag="kT")
                if m == 0:
                    # split so early row-tiles don't wait on the full load
                    nc.scalar.dma_start(kT[:, 0:256], kT_d[m][:, 0:256])
                    nc.scalar.dma_start(kT[:, 256:1024], kT_d[m][:, 256:1024])
                else:
                    nc.gpsimd.dma_start(kT[:], kT_d[m])
                va = io.tile([128, NT * 65], BF16, tag="va")
                nc.gpsimd.dma_start(
                    va[:].rearrange("p (a d) -> p a d", a=NT),
                    va_d[m].rearrange("(a b) d -> b a d", a=NT),
                )

                oT_fin = fp.tile([65, N], F32, tag="ofin")

                for g in range(NT // 4):  # row-tile quads
                    scoreT = scp.tile([128, 4096], BF16, tag="scoreT")
                    for t in range(4):
                        I = 4 * g + t
                        u0 = 896 - 128 * I
                        W = 128 * (I + 1)  # causal row width

                        # --- m2r band (upper PE half): m2r[ii, u-u0] ---
                        m2r = m2rp.tile([128, SROW], BF16, tag="m2r")
                        for ci, (c, wd) in enumerate(M2R_CHUNKS[I]):
                            pm = ps_mr.tile([128, 512], F32, tag="mr")
                            nc.tensor.matmul(
                                pm[:, :wd],
                                qT[64:128, 128 * I : 128 * (I + 1)],
                                rpeR[64:128, u0 + c : u0 + c + wd],
                                start=True,
                                stop=True,
                                tile_position=(64, 0) if ROW_PACK else None,
                            )
                            # evac psum -> sbuf (casts to bf16)
                            if ci % 2 == 0:
                                nc.scalar.copy(m2r[:, c : c + wd], pm[:, :wd])
                            else:
                                nc.vector.tensor_copy(m2r[:, c : c + wd], pm[:, :wd])

                        # full-width shear read of the bias row (one DMA per
                        # row tile, on the otherwise-idle sync queue)
                        bias = bp.tile([128, 1024], BF16, tag="bias")
                        nc.sync.dma_start(
                            bias[:, :W],
                            _shear_ap(m2r[:], SROW, 127, W),
                        )

                        # --- score row: w = (s + 1) + bias ---
                        wrow = wp.tile([128, N], BF16, tag="wrow")
                        for c, wd in S_CHUNKS[I]:
                            psz = ps_s.tile([128, 512], F32, tag="mm")
                            nc.tensor.matmul(
                                psz[:, :wd],
                                qT[0:64, 128 * I : 128 * (I + 1)],
                                kT[:, c : c + wd],
                                start=True,
                                stop=True,
                                tile_position=(0, 0) if ROW_PACK else None,
                            )
                            nc.vector.scalar_tensor_tensor(
                                wrow[:, c : c + wd],
                                psz[:, :wd],
                                1.0,
                                bias[:, c : c + wd],
                                mybir.AluOpType.add,
                                mybir.AluOpType.add,
                            )

                        # causal mask on the diagonal block: keep jj <= ii
                        nc.gpsimd.affine_select(
                            wrow[:, 128 * I : 128 * (I + 1)],
                            wrow[:, 128 * I : 128 * (I + 1)],
                            pattern=[[-1, 128]],
                            compare_op=mybir.AluOpType.is_ge,
                            fill=0.0,
                            base=0,
                            channel_multiplier=1,
                        )

                        # --- transpose 128-blocks, square-evac to scoreT ---
                        # up to 8 transpose blocks share one 1024-wide bf16
                        # psum bank; one Square evac per bank.
                        for c0 in range(0, W, 1024):
                            wd = min(1024, W - c0)
                            pw = ps_wt.tile([128, 1024], BF16, tag="wt")
                            for bofs in range(0, wd, 128):
                                nc.tensor.transpose(
                                    pw[:, bofs : bofs + 128],
                                    wrow[:, c0 + bofs : c0 + bofs + 128],
                                    idn[:],
                                )
                            nc.scalar.activation(
                                scoreT[:, 1024 * t + c0 : 1024 * t + c0 + wd],
                                pw[:, :wd],
                                mybir.ActivationFunctionType.Square,
                            )

                        # zero the never-written (j > diag) tail of this tile
                        # up to the quad's J extent
                        need = 128 * (4 * g + 4)
                        if W < need:
                            nc.vector.memset(
                                scoreT[:, 1024 * t + W : 1024 * t + need].bitcast(F32),
                                0.0,
                            )

                    # --- oT accumulation over J (512-wide moving) ---
                    pot = ps_ot.tile([65, 512], F32, tag="ot")
                    njs = 4 * g + 4
                    for J in range(njs):
                        rhs = _ap(
                            scoreT[:],
                            [[4096, 128], [1024, 4], [1, 128]],
                            offset=128 * J,
                        )
                        out3 = _ap(pot[:], [[512, 65], [128, 4], [1, 128]])
                        nc.tensor.matmul(
                            out3,
                            va[:, 65 * J : 65 * (J + 1)],
                            rhs,
                            start=(J == 0),
                            stop=(J == njs - 1),
                        )

                    # evac raw numerator/denominator rows; host finishes.
                    nc.scalar.copy(oT_fin[:, 512 * g : 512 * (g + 1)], pot[:])

                nc.sync.dma_start(oT_d[m], oT_fin[:])

    nc.compile()
    return nc


_NC_CACHE = {}


def get_program():
    if "nc" not in _NC_CACHE:
        _NC_CACHE["nc"] = build_program()
    return _NC_CACHE["nc"]


def prepare_inputs(q, k, v, rpe_matrix):
    """Host-side prep: returns per-core input maps."""
    q = np.asarray(q, dtype=np.float32).reshape(NBH, N, D)
    k = np.asarray(k, dtype=np.float32).reshape(NBH, N, D)
    v = np.asarray(v, dtype=np.float32).reshape(NBH, N, D)
    rpe = np.asarray(rpe_matrix, dtype=np.float32)

    BF = ml_dtypes.bfloat16
    qT = np.ascontiguousarray(q.transpose(0, 2, 1)).astype(BF)  # [32, 64, 1024]
    kT = np.ascontiguousarray(k.transpose(0, 2, 1)).astype(BF)
    va = np.concatenate([v, np.ones((NBH, N, 1), np.float32)], axis=2).astype(
        BF
    )  # [32,1024,65]

    # reversed rpe band: rpeR[:, u] = rpe[2046 - u] for u in [0, 1151)
    rpeR = np.zeros((64, RPE_W), np.float32)
    rpeR[:, :1151] = rpe[2046:895:-1].T
    rpeR = rpeR.astype(BF)
    idn = np.eye(128, dtype=BF)

    in_maps = []
    for c in range(N_CORES):
        sl = slice(c * BH_PER_CORE, (c + 1) * BH_PER_CORE)
        in_maps.append(
            {
                "qT": np.ascontiguousarray(qT[sl]),
                "kT": np.ascontiguousarray(kT[sl]),
                "va": np.ascontiguousarray(va[sl]),
                "rpeR": rpeR,
                "idn": idn,
            }
        )
    return in_maps, va


def run(q, k, v, rpe_matrix, trace=False):
    nc = get_program()
    in_maps, va = prepare_inputs(q, k, v, rpe_matrix)
    res = run_bass_kernel_spmd(nc, in_maps, list(range(N_CORES)), trace=trace)
    outs = [res.results[c]["oT"] for c in range(N_CORES)]
    oT = np.concatenate(outs, axis=0).astype(np.float64)  # [32, 65, 1024]
    # host epilogue: add the a0=1 prefix term (cumsum of [v, 1]), divide.
    corr = np.cumsum(va.astype(np.float64), axis=1)  # [32, 1024, 65]
    num = oT[:, :64, :].transpose(0, 2, 1) + corr[:, :, :64]
    den = oT[:, 64, :] + corr[:, :, 64]
    o = (num / den[:, :, None]).astype(np.float32)
    return o.reshape(B, H, N, D), res


def kernel(q, k, v, drop_noise=None, rpe_matrix=None, p=2, **kw):
    o, _ = run(q, k, v, rpe_matrix)
    return o


if __name__ == "__main__":
    rng = np.random.default_rng(0)
    q = rng.standard_normal((B, H, N, D), dtype=np.float32)
    k = rng.standard_normal((B, H, N, D), dtype=np.float32)
    v = rng.standard_normal((B, H, N, D), dtype=np.float32)
    rpe = rng.standard_normal((2 * N - 1, D), dtype=np.float32)
    o, _ = run(q, k, v, rpe)
    print("out", o.shape, o.dtype, np.abs(o).max())
